# revision 3
# baseline (speedup 1.0000x reference)
"""DualMem retrieval kernel for Trainium2 (8 NeuronCores, Bass/Tile).

Math (per reference):
    sim[b,c,m]  = <img[b], mem[c,m]>
    w           = exp(-beta * (1 - sim))
    adapt[b,c]  = sum_m mem[c,m] * w[b,c,m]
    logits[b,c] = 100 * <img[b], adapt[b,c] / ||adapt[b,c]||>

Algebraic reduction (avoids materializing adapt [B,C,D]):
    numer[b,c]  = sum_m w[b,c,m] * sim[b,c,m]
    denom[b,c]  = w^T G_c w,  G_c = mem_c @ mem_c^T  (11x11 Gram)
    logits      = 100 * numer / sqrt(denom)

Sharding: classes C=1000 split 125 per core across 8 cores.

Implementation notes (cost-model driven):
  * sim/Gram matmuls run in fp8e4m3 DoubleRow mode ([128, 2, cols] k-tile
    pairs contract 256 d per matmul at 0.5 cyc/col -- 4x under bf16).
    Inputs are pre-scaled by S=16 (folds into exp bias/scale + final bias).
  * Precision: plain fp8 sim loses ~2.9e-2 rel err through the exp(beta*.)
    amplification.  Shipping fp8 *residuals* for img (all 1024 d, cheap)
    and for the first 512 d of the memory bank brings it to ~1.4e-2:
    sim = mhi*(ihi+ilo) + mlo*ihi accumulated in one PSUM group (10 DR
    matmuls/group).  The Gram stays hi-only (verified irrelevant to err).
  * The bank is transposed + fp8-packed on the host into the exact SBUF
    layout ([128, 6, 2, cols]; rows 0-3 hi with d = 256Q+128i+p, rows 4-5
    lo for d < 512), so the device does plain 360 GB/s DMAs, no xbar
    transposes.  7 column slices stream on the otherwise-idle SP queue
    (DMA configs on a compute engine's sequencer would stall its ops);
    imgT (hi|lo) + bf16 consts (mask|em) issue from the ACT sequencer
    (idle until the first exp) so the SP sequencer reaches slice 1's
    config before slice 0's transfer ends.
  * PE order: one tiny junk matmul at t~0.8us latches the p-state ramp
    (full 2.4 GHz from ~3.8us, gaps don't reset it), then ALL sim/Gram
    matmuls in slice-arrival order, then the cross-block downstream, so
    the in-order PE queue never stalls on ACT/DVE latencies.
  * Downstream per block (one block per slice, [2,...,2,1,1] groups):
    exp on ACT, masked-Gram copy on DVE (GPSIMD cannot touch PSUM),
    u = G^T w on PE, [w*sim | w*u] broadcast-mul on DVE, then per group
    two 16-col nd matmuls (wsq_slice^T @ em) landing numer/denom b-major
    on 64 partitions -- finals (Ln/Exp, one shared ACT table;
    100/sqrt(d) = exp(-.5 ln d + ln(100/S))) and the final mul touch only
    [64, <=128] tiles and the output leaves as [64, 192] f32 rows of
    768B.  Emission order is tuned against the in-order engine queues
    (heads right after their block's sims, wsq of the previous block
    next, finals off the tail).
"""

import sys

sys.path.insert(0, "/opt/trn_rl_repo")

import ml_dtypes
import numpy as np

B, C, M, D = 64, 1000, 11, 1024
BETA = 5.5
N_CORES = 8
C_PER = C // N_CORES          # 125 classes per core
CPG = 11                      # classes per group
NG = 12                       # groups per core (11 full + one 44-row)
PG = CPG * M                  # 121 rows per full group
ROWS = 11 * PG + 4 * M        # 1375 class-major rows per core
S = 16.0                      # fp8 pre-scale (folded back out downstream)
NLO = 512                     # leading d-dims that ship an fp8 residual

# column slices of the group-major row space: (row offset, rows, padded
# width).  Widths are 16-aligned (DoubleRow ldweights requires the k-tile
# stride % 16 == 0).
SLICES = [(0, 242, 248), (242, 242, 248), (484, 242, 248), (726, 242, 248),
          (968, 242, 248), (1210, 121, 128), (1331, 44, 48)]
NCOL8 = 12 * sum(w for _, _, w in SLICES)
# blocks of groups sharing one downstream pass: one block per DMA slice,
# so each block's downstream starts as soon as its slice lands and the
# DVE chain (masks+wsq muls, all PSUM-bound) paces with the stream.
BLKS = [(0, 2), (2, 2), (4, 2), (6, 2), (8, 2), (10, 1), (11, 1)]
GW = [PG] * 11 + [4 * M]      # rows per group

_cache = {}


def _build():
    import concourse.mybir as mybir
    import concourse.tile as tile
    from concourse import bacc

    # Pin every activation to the one ACT table that holds BOTH Exp and Ln
    # (indices preserved -- empty the other sets instead of dropping them)
    # so the function table is loaded once and never swapped.
    if not getattr(bacc, "_act_tables_pinned", False):
        real = bacc.get_activation_tables

        def pinned(arch):
            return {k: (v if k == "natural_log_exp_and_others" else set())
                    for k, v in real(arch).items()}
        bacc.get_activation_tables = pinned
        bacc._act_tables_pinned = True

    f32 = mybir.dt.float32
    bf16 = mybir.dt.bfloat16
    fp8 = mybir.dt.float8e4
    DR = mybir.MatmulPerfMode.DoubleRow

    nc = bacc.Bacc("TRN2", target_bir_lowering=False, debug=False,
                   num_devices=N_CORES)

    mem8 = nc.dram_tensor("mem8", [128, NCOL8], fp8, kind="ExternalInput")
    img8 = nc.dram_tensor("img8", [128, 4 * 2 * 128], fp8,
                          kind="ExternalInput")
    constb = nc.dram_tensor("constb", [128, 137], bf16, kind="ExternalInput")
    out = nc.dram_tensor("out", [64, NG * 16], f32, kind="ExternalOutput")

    with tile.TileContext(nc) as tc:
        with (
            tc.tile_pool(name="const", bufs=1) as const,
            tc.tile_pool(name="sb", bufs=3) as sb,
            tc.tile_pool(name="ps_su", bufs=3, space="PSUM") as ps_su,
            tc.tile_pool(name="ps_g", bufs=3, space="PSUM") as ps_g,
            tc.tile_pool(name="ps_nd", bufs=1, space="PSUM") as ps_nd,
        ):
            # --- tiles ---
            mt = [const.tile([128, 2, 6, w], fp8, name=f"mt{si}",
                             tag=f"mt{si}")
                  for si, (o, r, w) in enumerate(SLICES)]
            it = const.tile([128, 4, 2, 128], fp8)   # [imgT hi | imgT lo]
            cb = const.tile([128, 137], bf16)
            lg = const.tile([64, NG * 16], f32)
            junk = const.tile([128, 16], fp8)
            bias_exp = const.tile([128, 1], f32)
            bias_eps = const.tile([64, 1], f32)
            bias_lns = const.tile([64, 1], f32)
            nc.vector.memset(junk[:], 0)
            nc.vector.memset(bias_exp[:], -BETA)
            nc.vector.memset(bias_eps[:], 1e-30)
            nc.vector.memset(bias_lns[:], float(np.log(100.0 / S)))

            nd = ps_nd.tile([64, 2 * NG * 16], f32, name="nd")
            # per-block su/gp PSUM tiles from 3-slot pools (3+3+1 banks):
            # u/wsq are emitted early so B0/B1's banks recycle in time for
            # the late small blocks.
            bloc = {}

            def get_block_tiles(nb):
                gn = BLKS[nb][1]
                su = ps_su.tile([128, gn * 128], f32, tag="su",
                                name=f"su{nb}")
                gp = ps_g.tile([128, gn * 128], f32, tag="gp",
                               name=f"gp{nb}")
                bloc[nb] = (su, gp, 0)
                return su, gp

            # PE p-state warm-up: one junk matmul sets pe_busy_start; the
            # model runs full-clock 3us later regardless of gaps.  Scribbles
            # on nd, which is rewritten (start=True) by the real nd matmuls.
            nc.tensor.matmul(nd[0:16, 0:16], junk[:], junk[:],
                             start=True, stop=True, skip_group_check=True)

            # --- input DMAs, all on the otherwise-idle SP queue so no
            # compute engine's sequencer is blocked by DMA configs; the
            # transfers serialize on DMA_ENGINES in this order ---
            c0 = 0
            srcs = []
            for si, (o, r, w) in enumerate(SLICES):
                srcs.append(mem8.ap()[:, c0:c0 + 12 * w].rearrange(
                    "p (i q c) -> p i q c", i=2, q=6))
                c0 += 12 * w
            nc.sync.dma_start(mt[0][:], srcs[0])
            # img/const configs ride the ACT sequencer (idle until the first
            # exp at ~4.8us) so the SP sequencer reaches slice 1's config
            # before slice 0's transfer ends -- no stream gap.
            nc.scalar.dma_start(
                it[:], img8.ap()[:, :].rearrange("p (q i c) -> p q i c",
                                                 q=4, i=2))
            nc.scalar.dma_start(cb[:], constb.ap()[:, :])
            for si in range(1, 7):
                nc.sync.dma_start(mt[si][:], srcs[si])

            mask = cb[0:128, 0:121]
            em = cb[0:128, 121:137]

            def gloc(g):
                if g < 10:
                    return g // 2, 121 * (g % 2)
                return (5, 0) if g == 10 else (6, 0)

            def blkq(g, q):
                """weights AP [p, 2, rows] for group g, 256-d chunk q
                (q in 0..3 = hi, 4..5 = lo of d<512)."""
                si, c0 = gloc(g)
                return mt[si][:, :, q, c0:c0 + GW[g]]

            # --- all sim/Gram matmuls first, in slice arrival order ---
            def emit_sims(g):
                k, gw = g, GW[g]
                nb = next(i for i, (g0, gn) in enumerate(BLKS)
                          if g0 <= g < g0 + gn)
                if g == BLKS[nb][0]:
                    get_block_tiles(nb)
                su, gp, co = bloc[nb]
                kk = g - BLKS[nb][0]
                sc = co + kk * 128
                # 10 accumulating DR matmuls: 4 hi*ihi + 4 hi*ilo + 2 lo*ihi
                steps = [(q, it[:, q, :, 0:64]) for q in range(4)] + \
                        [(q, it[:, q, :, 64:128]) for q in range(4)] + \
                        [(4 + q, it[:, q, :, 0:64]) for q in range(2)]
                for j, (q, ims) in enumerate(steps):
                    nc.tensor.matmul(su[0:gw, sc:sc + 64], blkq(g, q), ims,
                                     start=(j == 0), stop=(j == len(steps) - 1),
                                     perf_mode=DR, skip_group_check=True)
                for q in range(4):
                    blk = blkq(g, q)
                    nc.tensor.matmul(gp[0:gw, sc:sc + gw], blk, blk,
                                     start=(q == 0), stop=(q == 3),
                                     perf_mode=DR, skip_group_check=True)

            # --- downstream ---
            w4s, gm4s, sups = {}, {}, {}

            def emit_head(nb):
                """exp (ACT) + masked-Gram copy (GPSIMD) for a block."""
                g0, gn = BLKS[nb]
                gw = GW[g0]
                su, gp, co = bloc[nb]
                su = su[0:gw, co:co + gn * 128]
                gp = gp[0:gw, co:co + gn * 128]
                sups[nb] = su
                su4 = su.rearrange("p (k t b) -> p k t b", k=gn, t=2)
                w4 = sb.tile([128, gn * 64], bf16, tag="w4",
                             name=f"w4_{nb}")[0:gw]
                w4s[nb] = w4
                nc.scalar.activation(
                    w4.rearrange("p (k b) -> p k b", k=gn), su4[:, :, 0, :],
                    mybir.ActivationFunctionType.Exp,
                    bias=bias_exp[0:gw], scale=BETA / (S * S))
                gm4 = sb.tile([128, gn * 128], bf16, tag="gm4",
                              name=f"gm4_{nb}")[0:gw]
                gm4s[nb] = gm4
                gm4r = gm4.rearrange("p (k j) -> p k j", k=gn)[:, :, 0:gw]
                gp4 = gp.rearrange("p (k j) -> p k j", k=gn)[:, :, 0:gw]
                mb = mask[0:gw, 0:gw].rearrange("p (u j) -> p u j", u=1) \
                    .to_broadcast((gw, gn, gw))
                nc.vector.tensor_mul(gm4r, gp4, mb)

            def emit_u(nb):
                g0, gn = BLKS[nb]
                gw = GW[g0]
                su, w4, gm4 = sups[nb], w4s[nb], gm4s[nb]
                for k in range(gn):
                    nc.tensor.matmul(su[:, k * 128 + 64:(k + 1) * 128],
                                     gm4[:, k * 128:k * 128 + gw],
                                     w4[:, k * 64:(k + 1) * 64],
                                     start=True, stop=True,
                                     skip_group_check=True)

            def emit_nd(nb):
                g0, gn = BLKS[nb]
                gw = GW[g0]
                su, w4 = sups[nb], w4s[nb]
                su4 = su.rearrange("p (k t b) -> p k t b", k=gn, t=2)
                wsq = sb.tile([128, gn * 128], bf16, tag="wsq",
                              name=f"wsq_{nb}")[0:gw]
                wq4 = wsq.rearrange("p (k t b) -> p k t b", k=gn, t=2)
                w4b = w4.rearrange("p (k u b) -> p k u b", k=gn, u=1) \
                    .to_broadcast((gw, gn, 2, 64))
                nc.vector.tensor_mul(wq4, su4, w4b)
                for k in range(gn):
                    g = g0 + k
                    nc.tensor.matmul(nd[0:64, 16 * g:16 * g + 16],
                                     wsq[:, k * 128:k * 128 + 64],
                                     em[0:gw, :], start=True, stop=True,
                                     skip_group_check=True)
                    nc.tensor.matmul(nd[0:64, 192 + 16 * g:192 + 16 * g + 16],
                                     wsq[:, k * 128 + 64:(k + 1) * 128],
                                     em[0:gw, :], start=True, stop=True,
                                     skip_group_check=True)

            r_hs = {}

            def emit_final_lnexp(fp, c0, c1):
                """r = exp(-.5*ln(nd_d) + ln(100/S)) over cols c0:c1."""
                n = c1 - c0
                s_h = sb.tile([64, n], f32, tag=f"s{fp}", name=f"s_{fp}")
                nc.scalar.activation(s_h[:], nd[0:64, 192 + c0:192 + c1],
                                     mybir.ActivationFunctionType.Ln,
                                     bias=bias_eps[:], scale=1.0)
                r_h = sb.tile([64, n], f32, tag=f"r{fp}", name=f"r_{fp}")
                nc.scalar.activation(r_h[:], s_h[:],
                                     mybir.ActivationFunctionType.Exp,
                                     bias=bias_lns[:], scale=-0.5)
                r_hs[fp] = r_h

            def emit_final_mul(fp, c0, c1):
                nc.vector.tensor_mul(lg[:, c0:c1], nd[0:64, c0:c1],
                                     r_hs[fp][:])

            # Emission order drives each engine's in-order queue; every op
            # is emitted roughly when its inputs become available so no
            # queue head-blocks a ready successor.  Heads (exp on ACT,
            # mask on DVE) go out right after their block's sims; finals
            # (ACT Ln/Exp + DVE mul) go last so late-block heads are not
            # stuck behind them.
            arrivals = list(range(7))
            for k, nb in enumerate(arrivals):
                g0, gn = BLKS[nb]
                for g in range(g0, g0 + gn):
                    emit_sims(g)
                emit_head(nb)
                if k >= 1:
                    emit_nd(arrivals[k - 1])
                    if arrivals[k - 1] == 3:
                        # groups 0-7 finalized + shipped early
                        emit_final_lnexp(0, 0, 128)
                        emit_final_mul(0, 0, 128)
                        nc.sync.dma_start(out.ap()[:, 0:128],
                                          lg[0:64, 0:128])
                emit_u(nb)
            emit_nd(arrivals[-1])
            emit_final_lnexp(1, 128, 192)
            emit_final_mul(1, 128, 192)
            nc.sync.dma_start(out.ap()[:, 128:192], lg[0:64, 128:192])

    nc.compile()
    return nc


def _get_nc():
    if "nc" not in _cache:
        _cache["nc"] = _build()
    return _cache["nc"]


def _prep_inputs(img_features, memorized_image_feat):
    """Host-side formatting: scale, fp8 hi+lo cast, d-major chunk layout."""
    f8 = ml_dtypes.float8_e4m3
    bf = ml_dtypes.bfloat16

    def q8(x):
        return np.asarray(x, dtype=np.float32).astype(f8)

    imgT = np.ascontiguousarray(img_features.T * S)          # [1024, 64] f32
    ihi = q8(imgT)
    ilo = q8(imgT - ihi.astype(np.float32))

    def chunked(x):  # [1024, n] -> [128, 4, 2, n] with d = 256Q + 128i + p
        n = x.shape[1]
        return x.reshape(4, 2, 128, n).transpose(2, 0, 1, 3)

    imgb = np.concatenate([chunked(ihi), chunked(ilo)], axis=3)  # [128,4,2,128]

    maskem = np.zeros((128, 137), bf)
    for c in range(CPG):
        maskem[c * M:(c + 1) * M, c * M:(c + 1) * M] = 1.0
        maskem[c * M:(c + 1) * M, 121 + c] = 1.0

    in_maps = []
    for k in range(N_CORES):
        rows = memorized_image_feat[k * C_PER:(k + 1) * C_PER] \
            .reshape(ROWS, D)
        memT = rows.T * S                                    # [1024, 1375]
        mhi = q8(memT)
        mlo = q8(memT[:NLO] - mhi[:NLO].astype(np.float32))  # [512, 1375]
        hiQ = chunked(mhi)                                   # [128,4,2,1375]
        loQ = mlo.reshape(2, 2, 128, ROWS).transpose(2, 0, 1, 3)
        allQ = np.concatenate([hiQ, loQ], axis=1)            # [128,6,2,1375]
        allQ = allQ.transpose(0, 2, 1, 3)                    # [128,2,6,1375]
        buf = np.zeros((128, NCOL8), f8)
        off = 0
        for o, r, w in SLICES:
            s = allQ[:, :, :, o:o + r]
            if w > r:
                s = np.concatenate(
                    [s, np.zeros((128, 2, 6, w - r), f8)], axis=3)
            buf[:, off:off + 12 * w] = s.reshape(128, -1)
            off += 12 * w
        in_maps.append({"mem8": buf, "img8": imgb.reshape(128, -1),
                        "constb": maskem})
    return in_maps


def _gather(results):
    logits = np.empty((B, C), np.float32)
    for k in range(N_CORES):
        o = results[k]["out"].reshape(B, NG, 16)             # [64, 12, 16]
        full = o[:, :11, :CPG].reshape(B, 11 * CPG)          # [64, 121]
        logits[:, k * C_PER:k * C_PER + 121] = full
        logits[:, k * C_PER + 121:(k + 1) * C_PER] = o[:, 11, :4]
    return logits


def kernel(img_features, memorized_image_feat):
    from concourse.bass_utils import run_bass_kernel_spmd

    nc = _get_nc()
    in_maps = _prep_inputs(img_features, memorized_image_feat)
    res = run_bass_kernel_spmd(nc, in_maps, core_ids=list(range(N_CORES)))
    return _gather(res.results)
